# revision 1
# baseline (speedup 1.0000x reference)
"""Trainium2 Bass kernel for FGAEmbedder (B=32, T=1024, IN=1536, D=768).

Math (identical to the reference up to float reassociation):
    h  = relu(x @ W1^T + b1)           [B,T,IN]
    u  = h @ W2^T + b2                 [B,T,D]
    e  = relu(u @ We^T + be)
    un = e @ Wr^T + br                 [B,T]
    xe = u @ Wx^T + bx ; ye = u @ Wy^T + by
    pw[t] = mean_s cos(xe[t], ye[s]) = (xe[t] . ybar) / ||xe[t]||,
            ybar = mean_s ye[s]/||ye[s]||      (the TxT matrix never exists)
    out = sum_t softmax(rw0*un + rw1*pw)[t] * u[t]

Sharding: data-parallel over batch, 4 batches per core, weights replicated,
no collectives.  Activations are kept feature-major on chip ([feat, tok]);
x is pre-transposed and pre-cast to fp16 on the host so the device does
pure fp16 matmul work (f32 PSUM accumulation).

The per-batch reduction tail (norms -> softmax -> weighted sum) is
software-pipelined into the next batch's matmul phases: the PE is in-order,
so every PE op that consumes a DVE/ACT chain result is emitted behind a
dense block of matmuls that hides the chain latency.
"""

import numpy as np

import concourse.bass as bass
import concourse.bacc as bacc
import concourse.mybir as mybir
import concourse.tile as tile
from concourse.bass_utils import run_bass_kernel_spmd

B, T, IN, D = 32, 1024, 1536, 768
NCORES = 8
BPC = B // NCORES        # batches per core
NT = 512                 # token tile (matmul moving free dim)
NTT = T // NT            # token tiles per batch
KI = IN // 128           # 12 feature tiles of the 1536 dim
KD = D // 128            # 6 feature tiles of the 768 dim

F16 = mybir.dt.float16
F32 = mybir.dt.float32
AF = mybir.ActivationFunctionType
ALU = mybir.AluOpType
AX = mybir.AxisListType


def build_nc(bpc: int = BPC) -> bass.Bass:
    nc = bacc.Bacc()

    xt = nc.declare_dram_parameter("xt", [bpc, IN, T], F16, isOutput=False)
    w1t = nc.declare_dram_parameter("w1t", [IN, IN], F16, isOutput=False)
    w2t = nc.declare_dram_parameter("w2t", [IN, D], F16, isOutput=False)
    wet = nc.declare_dram_parameter("wet", [D, D], F16, isOutput=False)
    wxt = nc.declare_dram_parameter("wxt", [D, D], F16, isOutput=False)
    wyt = nc.declare_dram_parameter("wyt", [D, D], F16, isOutput=False)
    wrt = nc.declare_dram_parameter("wrt", [D, 1], F16, isOutput=False)
    b1d = nc.declare_dram_parameter("b1", [IN], F32, isOutput=False)
    b2d = nc.declare_dram_parameter("b2", [D], F32, isOutput=False)
    bed = nc.declare_dram_parameter("be", [D], F32, isOutput=False)
    bxd = nc.declare_dram_parameter("bx", [D], F32, isOutput=False)
    byd = nc.declare_dram_parameter("by", [D], F32, isOutput=False)
    # consts = [un_red_b*red_w0, red_w0, red_w1, 0]
    cst = nc.declare_dram_parameter("consts", [4], F32, isOutput=False)
    onesd = nc.declare_dram_parameter("onesv", [128, 1], F16, isOutput=False)
    onesr16 = nc.declare_dram_parameter("onesr16", [1, 128], F16, isOutput=False)
    out = nc.declare_dram_parameter("out", [bpc, D], F32, isOutput=True)

    with tile.TileContext(nc) as tc:
        _body(nc, tc, bpc, xt, w1t, w2t, wet, wxt, wyt, wrt,
              b1d, b2d, bed, bxd, byd, cst, onesd, onesr16, out)
    return nc


def _body(nc, tc, bpc, xt, w1t, w2t, wet, wxt, wyt, wrt,
          b1d, b2d, bed, bxd, byd, cst, onesd, onesr16, out):
    with (
        tc.tile_pool(name="wpool", bufs=1) as wpool,
        tc.tile_pool(name="u16p", bufs=2) as u16p,
        tc.tile_pool(name="bat", bufs=1) as bat,
        tc.tile_pool(name="xp", bufs=2) as xp,
        tc.tile_pool(name="hp", bufs=1) as hp,
        tc.tile_pool(name="ep", bufs=1) as ep,
        tc.tile_pool(name="yp", bufs=1) as yp,
        tc.tile_pool(name="sqp", bufs=12) as sqp,
        tc.tile_pool(name="tmpp", bufs=2) as tmpp,
        tc.tile_pool(name="rows", bufs=1) as rows,
        tc.tile_pool(name="rtmp", bufs=2) as rtmp,
        tc.tile_pool(name="bc16p", bufs=2) as bc16p,
        tc.tile_pool(name="mmp", bufs=3, space="PSUM") as mmp,
        tc.tile_pool(name="rpp", bufs=2, space="PSUM") as rpp,
        tc.tile_pool(name="bcp", bufs=2, space="PSUM") as bcp,
        tc.tile_pool(name="stkp", bufs=1, space="PSUM") as stkp,
    ):
        # ---- persistent weights / constants ----
        # first x tile goes out first so fc1 can start as soon as w1's
        # leading k-slices land; w1/w2 are split per k-tile for the same
        # reason (matmul k waits only on its own slice).
        first_xt = xp.tile([128, KI, NT], F16, tag="xt")
        x0r = xt[0].rearrange("(ko p) t -> p ko t", p=128)
        w1_sb = wpool.tile([128, KI, IN], F16)
        w1r = w1t.rearrange("(ko p) m -> p ko m", p=128)
        nc.sync.dma_start(first_xt[:, 0, :], x0r[:, 0, 0:NT])
        nc.sync.dma_start(w1_sb[:, 0, :], w1r[:, 0, :])
        b1_sb = wpool.tile([128, KI], F32)
        nc.sync.dma_start(b1_sb, b1d.rearrange("(o p) -> p o", p=128))
        for k in range(1, KI):
            nc.sync.dma_start(first_xt[:, k, :], x0r[:, k, 0:NT])
            nc.sync.dma_start(w1_sb[:, k, :], w1r[:, k, :])
        w2_sb = wpool.tile([128, KI, D], F16)
        w2r = w2t.rearrange("(ko p) m -> p ko m", p=128)
        for k in range(KI):
            nc.sync.dma_start(w2_sb[:, k, :], w2r[:, k, :])
        we_sb = wpool.tile([128, KD, D], F16)
        nc.sync.dma_start(we_sb, wet.rearrange("(ko p) m -> p ko m", p=128))
        wx_sb = wpool.tile([128, KD, D], F16)
        nc.sync.dma_start(wx_sb, wxt.rearrange("(ko p) m -> p ko m", p=128))
        wy_sb = wpool.tile([128, KD, D], F16)
        nc.sync.dma_start(wy_sb, wyt.rearrange("(ko p) m -> p ko m", p=128))
        wr_sb = wpool.tile([128, KD, 1], F16)
        nc.sync.dma_start(wr_sb, wrt.rearrange("(ko p) m -> p ko m", p=128))
        b2_sb = wpool.tile([128, KD], F32)
        nc.sync.dma_start(b2_sb, b2d.rearrange("(o p) -> p o", p=128))
        be_sb = wpool.tile([128, KD], F32)
        nc.sync.dma_start(be_sb, bed.rearrange("(o p) -> p o", p=128))
        bx_sb = wpool.tile([128, KD], F32)
        nc.sync.dma_start(bx_sb, bxd.rearrange("(o p) -> p o", p=128))
        by_sb = wpool.tile([128, KD], F32)
        nc.sync.dma_start(by_sb, byd.rearrange("(o p) -> p o", p=128))
        c_sb = wpool.tile([1, 4], F32)
        nc.sync.dma_start(c_sb, cst[None, :])
        ones_sb = wpool.tile([128, 1], F16)
        nc.sync.dma_start(ones_sb, onesd[:, :])
        onesr16_sb = wpool.tile([1, 128], F16)
        nc.sync.dma_start(onesr16_sb, onesr16[:, :])

        def alloc_batch(b):
            st = {"b": b}
            st["u16"] = u16p.tile([128, KD, T], F16, tag="u16", name=f"u16_{b}")
            st["xe16"] = bat.tile([128, KD, T], F16, tag="xe16", name=f"xe_{b}")
            st["ybp"] = bat.tile([128, KD, NTT], F32, tag="ybp", name=f"yp_{b}")
            st["invx"] = rows.tile([1, T], F32, tag="invx", name=f"ix_{b}")
            st["scores"] = rows.tile([1, T], F32, tag="scores", name=f"sc_{b}")
            st["ye"] = [None] * NTT
            return st

        def fc1_part(st, ti):
            b = st["b"]
            ns = slice(ti * NT, (ti + 1) * NT)
            if b == 0 and ti == 0:
                xt_sb = first_xt
            else:
                xt_sb = xp.tile([128, KI, NT], F16, tag="xt", name=f"xt{b}_{ti}")
                nc.sync.dma_start(
                    xt_sb,
                    xt[b].rearrange("(ko p) t -> p ko t", p=128)[:, :, ns])
            h = hp.tile([128, KI, NT], F16, tag="h", name=f"h{b}_{ti}")
            for m in range(KI):
                ps = mmp.tile([128, NT], F32, tag="mm")
                for k in range(KI):
                    nc.tensor.matmul(ps, w1_sb[:, k, m * 128:(m + 1) * 128],
                                     xt_sb[:, k, :],
                                     start=(k == 0), stop=(k == KI - 1))
                nc.scalar.activation(h[:, m, :], ps, AF.Relu,
                                     bias=b1_sb[:, m:m + 1])
            return h

        def fc2_part(st, ti, h):
            ns = slice(ti * NT, (ti + 1) * NT)
            for m in range(KD):
                ps = mmp.tile([128, NT], F32, tag="mm")
                for k in range(KI):
                    nc.tensor.matmul(ps, w2_sb[:, k, m * 128:(m + 1) * 128],
                                     h[:, k, :],
                                     start=(k == 0), stop=(k == KI - 1))
                nc.scalar.activation(st["u16"][:, m, ns], ps, AF.Identity,
                                     bias=b2_sb[:, m:m + 1])

        def une_part(st, ti):
            b = st["b"]
            ns = slice(ti * NT, (ti + 1) * NT)
            e = ep.tile([128, KD, NT], F16, tag="e", name=f"e{b}_{ti}")
            for m in range(KD):
                ps = mmp.tile([128, NT], F32, tag="mm")
                for k in range(KD):
                    nc.tensor.matmul(ps, we_sb[:, k, m * 128:(m + 1) * 128],
                                     st["u16"][:, k, ns],
                                     start=(k == 0), stop=(k == KD - 1))
                nc.scalar.activation(e[:, m, :], ps, AF.Relu,
                                     bias=be_sb[:, m:m + 1])
            # scores[ns] = rw0*un_pot = rw0*(Wr e) + rw0*br   (consts folded)
            rps = rpp.tile([1, NT], F32, tag="row")
            for k in range(KD):
                nc.tensor.matmul(rps, wr_sb[:, k, :], e[:, k, :],
                                 start=(k == 0), stop=(k == KD - 1))
            nc.scalar.activation(st["scores"][:, ns], rps, AF.Identity,
                                 bias=c_sb[:, 0:1], scale=c_sb[:, 1:2])

        def pwx_part(st, ti):
            b = st["b"]
            ns = slice(ti * NT, (ti + 1) * NT)
            xe16 = st["xe16"]
            sqs = []
            for m in range(KD):
                ps = mmp.tile([128, NT], F32, tag="mm")
                for k in range(KD):
                    nc.tensor.matmul(ps, wx_sb[:, k, m * 128:(m + 1) * 128],
                                     st["u16"][:, k, ns],
                                     start=(k == 0), stop=(k == KD - 1))
                nc.scalar.activation(xe16[:, m, ns], ps, AF.Identity,
                                     bias=bx_sb[:, m:m + 1])
                sq = sqp.tile([128, NT], F16, tag="sq", name=f"sx{b}_{ti}_{m}")
                nc.vector.tensor_mul(sq, xe16[:, m, ns], xe16[:, m, ns])
                sqs.append(sq)
            st["sqx"] = sqs

        def pwy_part(st, ti):
            b = st["b"]
            ns = slice(ti * NT, (ti + 1) * NT)
            ye = yp.tile([128, KD, NT], F16, tag="ye", name=f"ye{b}_{ti}")
            sqsy = []
            for m in range(KD):
                ps = mmp.tile([128, NT], F32, tag="mm")
                for k in range(KD):
                    nc.tensor.matmul(ps, wy_sb[:, k, m * 128:(m + 1) * 128],
                                     st["u16"][:, k, ns],
                                     start=(k == 0), stop=(k == KD - 1))
                nc.scalar.activation(ye[:, m, :], ps, AF.Identity,
                                     bias=by_sb[:, m:m + 1])
                sq = sqp.tile([128, NT], F16, tag="sq", name=f"sy{b}_{ti}_{m}")
                nc.vector.tensor_mul(sq, ye[:, m, :], ye[:, m, :])
                sqsy.append(sq)
            # paired sum-of-squares rows: ssy at col-group 0, ssx at col-group
            # 32 of one PSUM bank -- the per-pair matmuls run concurrently
            sqsx = st["sqx"]
            stk = stkp.tile([128, NT], F32, tag="stk")
            for m in range(KD):
                nc.tensor.matmul(stk[0:1, :], ones_sb, sqsy[m],
                                 start=(m == 0), stop=(m == KD - 1),
                                 tile_position=(0, 0))
                nc.tensor.matmul(stk[32:33, :], ones_sb, sqsx[m],
                                 start=(m == 0), stop=(m == KD - 1),
                                 tile_position=(0, 32))
            # invy chain (ACT/DVE only) stays here so it's ready well before
            # the deferred PE consumer (y_pe) issues.
            t1 = rtmp.tile([1, NT], F32, tag="rt")
            nc.scalar.activation(t1, stk[0:1, :], AF.Sqrt)
            nc.vector.tensor_scalar_max(t1, t1, 1e-12)
            nc.vector.reciprocal(t1, t1)
            t1h = rtmp.tile([1, NT], F16, tag="rth")
            nc.vector.tensor_copy(t1h, t1)
            # realign ssx (lane 32) to lane 0: sqrt to SBUF on its own lane,
            # then an SBUF->SBUF DMA hop (off the critical path; invx is only
            # needed in pass2_q)
            t32 = tmpp.tile([128, NT], F32, tag="tmp32")
            nc.scalar.activation(t32[32:33, :], stk[32:33, :], AF.Sqrt)
            sxr = rtmp.tile([1, NT], F32, tag="sxr")
            nc.sync.dma_start(sxr, t32[32:33, :])
            t0 = rtmp.tile([1, NT], F32, tag="rt")
            nc.vector.tensor_scalar_max(t0, sxr, 1e-12)
            nc.vector.reciprocal(t0, t0)
            # pre-scale by red_w1 so the exposed pass2 chain skips that op
            nc.vector.tensor_scalar_mul(st["invx"][:, ns], t0, c_sb[:, 2:3])
            st["ye"][ti] = ye
            st["t1h_%d" % ti] = t1h

        def y_pe(st, ti):
            # broadcast 1/||ye|| and reduce yn into ybar parts; emitted after
            # a dense matmul block so the PE never waits on the invy chain.
            ye = st["ye"][ti]
            ivb = bcp.tile([128, NT], F32, tag="bc")
            nc.tensor.matmul(ivb, onesr16_sb, st["t1h_%d" % ti],
                             start=True, stop=True)
            for m in range(KD):
                tmp = tmpp.tile([128, NT], F16, tag="tmp")
                nc.vector.tensor_mul(tmp, ye[:, m, :], ivb)
                nc.vector.reduce_sum(st["ybp"][:, m, ti:ti + 1], tmp, axis=AX.X)
            if ti == NTT - 1:
                b = st["b"]
                ybar16 = bat.tile([128, KD, 1], F16, tag="ybar", name=f"yb{b}")
                ybf = bat.tile([128, KD, 1], F32, tag="ybf", name=f"yf{b}")
                nc.vector.tensor_add(ybf, st["ybp"][:, :, 0:1],
                                     st["ybp"][:, :, 1:2])
                nc.vector.tensor_scalar_mul(ybar16, ybf, 1.0 / T)
                st["ybar16"] = ybar16

        def pass2_q(st):
            # q = xe . ybar ; scores += rw1 * q * invx ; softmax weights.
            # Per-ti partial maxes shorten the chain after the last q matmul.
            b = st["b"]
            mxp = rows.tile([1, NTT], F32, tag="mxp", name=f"mxp{b}")
            for ti in range(NTT):
                ns = slice(ti * NT, (ti + 1) * NT)
                qps = rpp.tile([1, NT], F32, tag="row")
                for k in range(KD):
                    nc.tensor.matmul(qps, st["ybar16"][:, k, :],
                                     st["xe16"][:, k, ns],
                                     start=(k == 0), stop=(k == KD - 1))
                s0 = rtmp.tile([1, NT], F32, tag="rt")
                nc.vector.tensor_mul(s0, qps, st["invx"][:, ns])
                nc.vector.tensor_add(st["scores"][:, ns], st["scores"][:, ns],
                                     s0)
                nc.vector.reduce_max(mxp[:, ti:ti + 1], st["scores"][:, ns],
                                     axis=AX.X)
            scores = st["scores"]
            mx = rows.tile([1, 1], F32, tag="mx", name=f"mx{b}")
            nc.vector.reduce_max(mx, mxp, axis=AX.X, negate=True)
            nc.scalar.activation(scores, scores, AF.Exp, bias=mx)
            ewh = rows.tile([1, T], F16, tag="ewh", name=f"ew{b}")
            nc.vector.tensor_copy(ewh, scores)
            st["ewh"] = ewh
            # 1/sum applied to the final [D] vector instead of the weights
            # row -- keeps the sum/reciprocal off the wbc critical path
            sm = rows.tile([1, 1], F32, tag="sm", name=f"sm{b}")
            nc.vector.reduce_sum(sm, scores, axis=AX.X)
            nc.vector.reciprocal(sm, sm)
            smh = rows.tile([1, 1], F16, tag="smh", name=f"sh{b}")
            nc.vector.tensor_copy(smh, sm)
            st["smh"] = smh

        def pass2_w(st, tail=False):
            # out[b] = sum_t w[t] * u[:, t].  In the pipeline the DVE does the
            # reduction (hidden under the next batch's matmuls); in the tail
            # the ACT does it via accum_out because the ACT is idle there and
            # the DVE chain would be exposed.
            b = st["b"]
            oacc = bat.tile([128, KD, NTT], F32, tag="oacc", name=f"oa{b}")
            for ti in range(NTT):
                ns = slice(ti * NT, (ti + 1) * NT)
                wbc = bcp.tile([128, NT], F32, tag="bc")
                nc.tensor.matmul(wbc, onesr16_sb, st["ewh"][:, ns],
                                 start=True, stop=True)
                if tail:
                    # alternate the 6 reductions between ACT (accum_out) and
                    # DVE so they drain concurrently in the exposed tail
                    wbc16 = bc16p.tile([128, NT], F16, tag="bc16")
                    nc.scalar.activation(wbc16, wbc, AF.Identity)
                    for m in range(KD):
                        tmp = tmpp.tile([128, NT], F16, tag="tmp")
                        nc.vector.tensor_mul(tmp, st["u16"][:, m, ns], wbc16)
                        if m % 2 == 0:
                            nc.scalar.activation(tmp, tmp, AF.Identity,
                                                 accum_out=oacc[:, m,
                                                               ti:ti + 1])
                        else:
                            nc.vector.reduce_sum(oacc[:, m, ti:ti + 1], tmp,
                                                 axis=AX.X)
                else:
                    for m in range(KD):
                        tmp = tmpp.tile([128, NT], F16, tag="tmp")
                        nc.vector.tensor_mul(tmp, st["u16"][:, m, ns], wbc)
                        nc.vector.reduce_sum(oacc[:, m, ti:ti + 1], tmp,
                                             axis=AX.X)
            smb = bcp.tile([128, 1], F32, tag="bc")
            nc.tensor.matmul(smb, onesr16_sb, st["smh"], start=True, stop=True)
            ofin = bat.tile([128, KD, 1], F32, tag="ofin", name=f"of{b}")
            nc.vector.tensor_add(ofin, oacc[:, :, 0:1], oacc[:, :, 1:2])
            nc.vector.tensor_scalar_mul(ofin, ofin, smb)
            nc.sync.dma_start(out[b].rearrange("(mo p) -> p mo", p=128),
                              ofin[:, :, 0])

        prev = None
        for b in range(bpc):
            st = alloc_batch(b)
            h0 = fc1_part(st, 0)
            if prev is not None:
                y_pe(prev, 1)            # chain computed during prev's pwy
            fc2_part(st, 0, h0)          # 72 MMs cover prev's yn/ybar DVE
            if prev is not None:
                pass2_q(prev)            # q MMs + softmax chain on DVE/ACT
            une_part(st, 0)              # 42 MMs cover the softmax chain
            pwx_part(st, 0)
            if prev is not None:
                pass2_w(prev)            # weight-bcast + weighted-sum DVE
            pwy_part(st, 0)
            h1 = fc1_part(st, 1)
            y_pe(st, 0)                  # chain from pwy(st,0) is long ready
            fc2_part(st, 1, h1)
            if b < bpc - 1:
                une_part(st, 1)
                pwx_part(st, 1)
                pwy_part(st, 1)
            else:
                # last batch: start the invy chain as early as possible and
                # hide its own reduction tail behind its remaining matmuls
                pwy_part(st, 1)
                une_part(st, 1)
                y_pe(st, 1)
                pwx_part(st, 1)
            prev = st
        pass2_q(prev)
        pass2_w(prev, tail=True)


_CACHE = {}


def _get_nc():
    if "nc" not in _CACHE:
        nc = build_nc(BPC)
        nc.finalize()
        _CACHE["nc"] = nc
    return _CACHE["nc"]


def make_in_maps(x, fc1_w, fc1_b, fc2_w, fc2_b, un_emb_w, un_emb_b,
                 un_red_w, un_red_b, pw_x_w, pw_x_b, pw_y_w, pw_y_b, red_w):
    shared = {
        "w1t": np.ascontiguousarray(fc1_w.T).astype(np.float16),
        "w2t": np.ascontiguousarray(fc2_w.T).astype(np.float16),
        "wet": np.ascontiguousarray(un_emb_w.T).astype(np.float16),
        "wxt": np.ascontiguousarray(pw_x_w.T).astype(np.float16),
        "wyt": np.ascontiguousarray(pw_y_w.T).astype(np.float16),
        "wrt": np.ascontiguousarray(un_red_w.T).astype(np.float16),
        "b1": np.asarray(fc1_b, np.float32),
        "b2": np.asarray(fc2_b, np.float32),
        "be": np.asarray(un_emb_b, np.float32),
        "bx": np.asarray(pw_x_b, np.float32),
        "by": np.asarray(pw_y_b, np.float32),
        "consts": np.array([un_red_b[0] * red_w[0], red_w[0], red_w[1], 0.0],
                           np.float32),
        "onesv": np.ones([128, 1], np.float16),
        "onesr16": np.ones([1, 128], np.float16),
    }
    in_maps = []
    for c in range(NCORES):
        xs = np.ascontiguousarray(
            x[c * BPC:(c + 1) * BPC].transpose(0, 2, 1)).astype(np.float16)
        in_maps.append({"xt": xs, **shared})
    return in_maps


def kernel(**inputs) -> np.ndarray:
    inputs = {k: np.asarray(v) for k, v in inputs.items()}
    nc = _get_nc()
    in_maps = make_in_maps(**inputs)
    res = run_bass_kernel_spmd(nc, in_maps, core_ids=list(range(NCORES)))
    return np.concatenate([res.results[c]["out"] for c in range(NCORES)], axis=0)



# revision 21
# speedup vs baseline: 1.2019x; 1.2019x over previous
"""Trainium2 Bass kernel for FGAEmbedder (B=32, T=1024, IN=1536, D=768).

Math (identical to the reference up to float reassociation + fp8 noise on
the score paths):
    h  = relu(x @ W1^T + b1)           [B,T,IN]   fp16 (exact path)
    u  = h @ W2^T + b2                 [B,T,D]    fp8 (scores only)
    e  = relu(u @ We^T + be)  ; un = e @ Wr^T + br       (score path)
    xe = u @ Wx^T + bx ; ye = u @ Wy^T + by              (score path)
    pw[t] = (xe[t] . ybar) / ||xe[t]||, ybar = mean_s ye[s]/||ye[s]||
    w  = softmax(rw0*un + rw1*pw)
    out = (sum_t w[t] * h[t]) @ W2^T + b2        <- exact: fc2 is linear, so
          the weighted sum is pushed through W2 in fp16 while the bulk fc2
          matmul runs in fp8.

Precision plan (validated vs the jax reference at rel_err ~8e-3 < 2e-2):
  - fc1 fp16 (feeds the exact output path through h).
  - fc2 / une / pwx / pwy matmuls in fp8e4 with DoubleRow perf mode
    (2 k-subtiles per instruction, ~1.8x tensor-engine throughput).
  - Activations quantized to fp8 at scale 16, weights at scale 64; the
    1/1024 product descale is folded into the PSUM-evacuation activations.
  - Final correction: V = sum_t w_t h_t (DVE, fp16), out = V @ (W2hi +
    W2lo/16) with W2 split into two fp8 planes so the exact path keeps
    ~fp16 weight precision without a 2.4MB fp16 W2 in SBUF.

Sharding: data-parallel over batch, 4 batches per core, no collectives.
"""

import numpy as np
import ml_dtypes

import concourse.bass as bass
import concourse.bacc as bacc
import concourse.mybir as mybir
import concourse.tile as tile
from concourse.bass_utils import run_bass_kernel_spmd

B, T, IN, D = 32, 1024, 1536, 768
NCORES = 8
BPC = B // NCORES        # batches per core
NT = 512                 # token tile (matmul moving free dim)
NTT = T // NT            # token tiles per batch
KI = IN // 128           # 12 feature tiles of the 1536 dim
KIP = KI // 2            # 6 fp8 double-row k-pairs
KD = D // 128            # 6 feature tiles of the 768 dim
KDP = KD // 2            # 3 fp8 double-row k-pairs

SX = 16.0                # fp8 activation scale
SW = 64.0                # fp8 weight scale

F16 = mybir.dt.float16
F8 = mybir.dt.float8e4
F32 = mybir.dt.float32
AF = mybir.ActivationFunctionType
ALU = mybir.AluOpType
AX = mybir.AxisListType
DR = mybir.MatmulPerfMode.DoubleRow


DEBUG = False


def build_nc(bpc: int = BPC) -> bass.Bass:
    nc = bacc.Bacc()

    xt = nc.declare_dram_parameter("xt", [bpc, IN, T], F16, isOutput=False)
    w1t = nc.declare_dram_parameter("w1t", [IN, IN], F16, isOutput=False)
    w2hi = nc.declare_dram_parameter("w2hi", [IN, D], F8, isOutput=False)
    w2lo = nc.declare_dram_parameter("w2lo", [IN, D], F8, isOutput=False)
    wet = nc.declare_dram_parameter("wet", [D, D], F8, isOutput=False)
    wxt = nc.declare_dram_parameter("wxt", [D, D], F8, isOutput=False)
    wyt = nc.declare_dram_parameter("wyt", [D, D], F8, isOutput=False)
    wrt = nc.declare_dram_parameter("wrt", [D, 1], F16, isOutput=False)
    b1d = nc.declare_dram_parameter("b1", [IN], F32, isOutput=False)
    b2sd = nc.declare_dram_parameter("b2s", [D], F32, isOutput=False)   # 16*b2
    bed = nc.declare_dram_parameter("be", [D], F32, isOutput=False)
    bxd = nc.declare_dram_parameter("bxs", [D], F32, isOutput=False)    # 16*bx
    byd = nc.declare_dram_parameter("bys", [D], F32, isOutput=False)    # 16*by
    # consts = [un_red_b*red_w0, red_w0, red_w1/1024, 0]
    cst = nc.declare_dram_parameter("consts", [4], F32, isOutput=False)
    onesd = nc.declare_dram_parameter("onesv", [128, 1], F16, isOutput=False)
    onesr16 = nc.declare_dram_parameter("onesr16", [1, 128], F16, isOutput=False)
    b2rep = nc.declare_dram_parameter("b2rep", [4, D], F16, isOutput=False)
    out = nc.declare_dram_parameter("out", [bpc, D], F32, isOutput=True)
    dbg = {}
    if DEBUG:
        dbg["h"] = nc.declare_dram_parameter("dbg_h", [128, KI, T], F16,
                                             isOutput=True)
        dbg["u8"] = nc.declare_dram_parameter("dbg_u8", [128, KD, NT], F8,
                                              isOutput=True)
        dbg["xe8"] = nc.declare_dram_parameter("dbg_xe8", [128, KD, T], F8,
                                               isOutput=True)
        dbg["ew"] = nc.declare_dram_parameter("dbg_ew", [4, T], F16,
                                              isOutput=True)
        dbg["v"] = nc.declare_dram_parameter("dbg_v", [128, KI, 4], F16,
                                             isOutput=True)
        dbg["sm"] = nc.declare_dram_parameter("dbg_sm", [4, 1], F32,
                                              isOutput=True)
        dbg["yb"] = nc.declare_dram_parameter("dbg_yb", [128, KDP, 2, 16], F8,
                                              isOutput=True)

    with tile.TileContext(nc) as tc:
        _body(nc, tc, bpc, xt, w1t, w2hi, w2lo, wet, wxt, wyt, wrt,
              b1d, b2sd, bed, bxd, byd, cst, onesd, onesr16, b2rep, out, dbg)
    return nc


def _body(nc, tc, bpc, xt, w1t, w2hi, w2lo, wet, wxt, wyt, wrt,
          b1d, b2sd, bed, bxd, byd, cst, onesd, onesr16, b2rep, out, dbg={}):
    with (
        tc.tile_pool(name="wpool", bufs=1) as wpool,
        tc.tile_pool(name="hpool", bufs=2) as hpool,
        tc.tile_pool(name="xp", bufs=2) as xp,
        tc.tile_pool(name="h8p", bufs=1) as h8p,
        tc.tile_pool(name="u8p", bufs=2) as u8p,
        tc.tile_pool(name="xe8p", bufs=2) as xe8p,
        tc.tile_pool(name="ye8p", bufs=2) as ye8p,
        tc.tile_pool(name="ep", bufs=3) as ep,
        tc.tile_pool(name="sqp", bufs=3) as sqp,
        tc.tile_pool(name="tmpp", bufs=2) as tmpp,
        tc.tile_pool(name="rows", bufs=1) as rows,
        tc.tile_pool(name="rtmp", bufs=2) as rtmp,
        tc.tile_pool(name="bat", bufs=1) as bat,
        tc.tile_pool(name="bc16p", bufs=2) as bc16p,
        tc.tile_pool(name="mmp", bufs=5, space="PSUM") as mmp,
        tc.tile_pool(name="rpp", bufs=2, space="PSUM") as rpp,
        tc.tile_pool(name="bcp", bufs=1, space="PSUM") as bcp,
    ):
        # ---- persistent weights / constants ----
        # first x tile + w1 k-slices go out first so fc1 starts streaming
        # as soon as its leading slices land.
        first_xt = xp.tile([128, KI, NT], F16, tag="xt")
        x0r = xt[0].rearrange("(ko p) t -> p ko t", p=128)
        w1_sb = wpool.tile([128, KI, IN], F16)
        w1r = w1t.rearrange("(ko p) m -> p ko m", p=128)
        # smallest-first: the m=0 stationary column block of each k plus the
        # k's x slice, so fc1's first PSUM group starts after ~500KB of DMA
        b1_sb = wpool.tile([128, KI], F32)
        nc.sync.dma_start(b1_sb, b1d.rearrange("(o p) -> p o", p=128))
        for k in range(KI):
            nc.sync.dma_start(w1_sb[:, k, 0:128], w1r[:, k, 0:128])
            nc.sync.dma_start(first_xt[:, k, :], x0r[:, k, 0:NT])
        for k in range(KI):
            nc.sync.dma_start(w1_sb[:, k, 128:IN], w1r[:, k, 128:IN])
        w2h_sb = wpool.tile([128, KI, D], F8)
        w2hr = w2hi.rearrange("(ko p) m -> p ko m", p=128)
        for k in range(KI):
            nc.sync.dma_start(w2h_sb[:, k, :], w2hr[:, k, :])
        we_sb = wpool.tile([128, KD, D], F8)
        nc.sync.dma_start(we_sb, wet.rearrange("(ko p) m -> p ko m", p=128))
        wx_sb = wpool.tile([128, KD, D], F8)
        nc.sync.dma_start(wx_sb, wxt.rearrange("(ko p) m -> p ko m", p=128))
        wy_sb = wpool.tile([128, KD, D], F8)
        nc.sync.dma_start(wy_sb, wyt.rearrange("(ko p) m -> p ko m", p=128))
        wr_sb = wpool.tile([128, KD, 1], F16)
        nc.sync.dma_start(wr_sb, wrt.rearrange("(ko p) m -> p ko m", p=128))
        b2s_sb = wpool.tile([128, KD], F32)
        nc.sync.dma_start(b2s_sb, b2sd.rearrange("(o p) -> p o", p=128))
        be_sb = wpool.tile([128, KD], F32)
        nc.sync.dma_start(be_sb, bed.rearrange("(o p) -> p o", p=128))
        bx_sb = wpool.tile([128, KD], F32)
        nc.sync.dma_start(bx_sb, bxd.rearrange("(o p) -> p o", p=128))
        by_sb = wpool.tile([128, KD], F32)
        nc.sync.dma_start(by_sb, byd.rearrange("(o p) -> p o", p=128))
        c_sb = wpool.tile([1, 4], F32)
        nc.sync.dma_start(c_sb, cst[None, :])
        ones_sb = wpool.tile([128, 1], F16)
        nc.sync.dma_start(ones_sb, onesd[:, :])
        onesr16_sb = wpool.tile([1, 128], F16)
        nc.sync.dma_start(onesr16_sb, onesr16[:, :])
        w2l_sb = wpool.tile([128, KI, D], F8)
        nc.sync.dma_start(w2l_sb, w2lo.rearrange("(ko p) m -> p ko m", p=128))
        b2r_sb = wpool.tile([4, D], F16)
        nc.sync.dma_start(b2r_sb, b2rep[:, :])

        # shared across batches: per-batch softmax 1/sum at partition b,
        # fp16 V (weighted H sums) and V/16 for the two-plane W2 matmul
        smcol = bat.tile([4, 1], F32, tag="smcol", name="smcol")
        v16 = bat.tile([128, KI, 4], F16, tag="v16", name="v16")
        v16l = bat.tile([128, KI, 4], F16, tag="v16l", name="v16l")

        def alloc_batch(b):
            st = {"b": b}
            st["h"] = hpool.tile([128, KI, T], F16, tag="h", name=f"h_{b}")
            st["xe8"] = xe8p.tile([128, KD, T], F8, tag="xe8", name=f"xe_{b}")
            st["ybp"] = bat.tile([128, KDP, 2, NTT], F32, tag="ybp",
                                 name=f"yp_{b}")
            st["invx"] = rows.tile([1, T], F32, tag="invx", name=f"ix_{b}")
            st["scores"] = rows.tile([1, T], F32, tag="scores", name=f"sc_{b}")
            st["ye8"] = [None] * NTT
            st["u8"] = [None] * NTT
            return st

        def fc1_part(st, ti):
            b = st["b"]
            ns = slice(ti * NT, (ti + 1) * NT)
            if b == 0 and ti == 0:
                xt_sb = first_xt
            else:
                xt_sb = xp.tile([128, KI, NT], F16, tag="xt", name=f"xt{b}_{ti}")
                nc.sync.dma_start(
                    xt_sb,
                    xt[b].rearrange("(ko p) t -> p ko t", p=128)[:, :, ns])
            h8 = h8p.tile([128, KI, NT], F8, tag="h8", name=f"h8{b}_{ti}")
            for m in range(KI):
                ps = mmp.tile([128, NT], F32, tag="mm")
                for k in range(KI):
                    nc.tensor.matmul(ps, w1_sb[:, k, m * 128:(m + 1) * 128],
                                     xt_sb[:, k, :],
                                     start=(k == 0), stop=(k == KI - 1))
                nc.scalar.activation(st["h"][:, m, ns], ps, AF.Relu,
                                     bias=b1_sb[:, m:m + 1])
                # fp8 copy (x16) for the fc2 double-row matmul
                nc.vector.tensor_scalar_mul(h8[:, m, :], st["h"][:, m, ns],
                                            SX)
            return h8

        def fc2_part(st, ti, h8):
            b = st["b"]
            u8 = u8p.tile([128, KD, NT], F8, tag="u8", name=f"u8{b}_{ti}")
            for m in range(KD):
                ps = mmp.tile([128, NT], F32, tag="mm")
                for kp in range(KIP):
                    nc.tensor.matmul(
                        ps, w2h_sb[:, 2 * kp:2 * kp + 2, m * 128:(m + 1) * 128],
                        h8[:, 2 * kp:2 * kp + 2, :],
                        start=(kp == 0), stop=(kp == KIP - 1), perf_mode=DR)
                # u8 = 16*u quantized: (psum*1024)/64 + 16*b2
                nc.scalar.activation(u8[:, m, :], ps, AF.Identity,
                                     bias=b2s_sb[:, m:m + 1], scale=1.0 / SW)
            st["u8"][ti] = u8
            if dbg and b == 0 and ti == 0:
                nc.sync.dma_start(dbg["u8"][:, :, :], u8)

        def une_part(st, ti):
            b = st["b"]
            ns = slice(ti * NT, (ti + 1) * NT)
            u8 = st["u8"][ti]
            # wr-row matmuls interleave 2 groups behind the une matmuls so
            # the PE never waits on the ACT/e chain; e tiles ring with 3 bufs
            es = {}
            rps = rpp.tile([1, NT], F32, tag="row")

            def wr_mm(k):
                nc.tensor.matmul(rps, wr_sb[:, k, :], es.pop(k),
                                 start=(k == 0), stop=(k == KD - 1))

            for m in range(KD):
                ps = mmp.tile([128, NT], F32, tag="mm")
                for kp in range(KDP):
                    nc.tensor.matmul(
                        ps, we_sb[:, 2 * kp:2 * kp + 2, m * 128:(m + 1) * 128],
                        u8[:, 2 * kp:2 * kp + 2, :],
                        start=(kp == 0), stop=(kp == KDP - 1), perf_mode=DR)
                e = ep.tile([128, NT], F16, tag="e", name=f"e{b}_{ti}_{m}")
                nc.scalar.activation(e, ps, AF.Relu,
                                     bias=be_sb[:, m:m + 1],
                                     scale=1.0 / (SX * SW))
                es[m] = e
                if m >= 2:
                    wr_mm(m - 2)
            wr_mm(KD - 2)
            wr_mm(KD - 1)
            # scores[ns] = rw0*un_pot = rw0*(Wr e) + rw0*br   (consts folded)
            nc.scalar.activation(st["scores"][:, ns], rps, AF.Identity,
                                 bias=c_sb[:, 0:1], scale=c_sb[:, 1:2])

        def pwx_part(st, ti):
            b = st["b"]
            ns = slice(ti * NT, (ti + 1) * NT)
            u8 = st["u8"][ti]
            xe8 = st["xe8"]
            sqs = {}
            rps = rpp.tile([1, NT], F32, tag="row")

            def ss_mm(m):
                nc.tensor.matmul(rps, ones_sb, sqs.pop(m),
                                 start=(m == 0), stop=(m == KD - 1))

            for m in range(KD):
                ps = mmp.tile([128, NT], F32, tag="mm")
                for kp in range(KDP):
                    nc.tensor.matmul(
                        ps, wx_sb[:, 2 * kp:2 * kp + 2, m * 128:(m + 1) * 128],
                        u8[:, 2 * kp:2 * kp + 2, :],
                        start=(kp == 0), stop=(kp == KDP - 1), perf_mode=DR)
                # xe8 = 16*xe quantized
                nc.scalar.activation(xe8[:, m, ns], ps, AF.Identity,
                                     bias=bx_sb[:, m:m + 1], scale=1.0 / SW)
                sq = sqp.tile([128, NT], F16, tag="sq", name=f"sx{b}_{ti}_{m}")
                nc.vector.tensor_mul(sq, xe8[:, m, ns], xe8[:, m, ns])
                sqs[m] = sq
                if m >= 2:
                    ss_mm(m - 2)
            ss_mm(KD - 2)
            ss_mm(KD - 1)
            # ssx = 256*||xe||^2 ; invx = rw1/1024 * 1/(16*||xe||)
            sx = rtmp.tile([1, NT], F32, tag="rt")
            nc.scalar.activation(sx, rps, AF.Sqrt)
            t0 = rtmp.tile([1, NT], F32, tag="rt")
            nc.vector.reciprocal_approx_fast(t0, sx)
            nc.vector.tensor_scalar_mul(st["invx"][:, ns], t0, c_sb[:, 2:3])

        def pwy_part(st, ti):
            b = st["b"]
            ns = slice(ti * NT, (ti + 1) * NT)
            u8 = st["u8"][ti]
            ye16 = ye8p.tile([128, KD, NT], F16, tag="ye16",
                             name=f"ye{b}_{ti}")
            sqs = {}
            rps = rpp.tile([1, NT], F32, tag="row")

            def ss_mm(m):
                nc.tensor.matmul(rps, ones_sb, sqs.pop(m),
                                 start=(m == 0), stop=(m == KD - 1))

            for m in range(KD):
                ps = mmp.tile([128, NT], F32, tag="mm")
                for kp in range(KDP):
                    nc.tensor.matmul(
                        ps, wy_sb[:, 2 * kp:2 * kp + 2, m * 128:(m + 1) * 128],
                        u8[:, 2 * kp:2 * kp + 2, :],
                        start=(kp == 0), stop=(kp == KDP - 1), perf_mode=DR)
                nc.scalar.activation(ye16[:, m, :], ps, AF.Identity,
                                     bias=by_sb[:, m:m + 1], scale=1.0 / SW)
                sq = sqp.tile([128, NT], F16, tag="sq", name=f"sy{b}_{ti}_{m}")
                nc.vector.tensor_mul(sq, ye16[:, m, :], ye16[:, m, :])
                sqs[m] = sq
                if m >= 2:
                    ss_mm(m - 2)
            ss_mm(KD - 2)
            ss_mm(KD - 1)
            # ssy = 256*||ye||^2 ; invy = 1/(16*||ye||) (fp16 for broadcast)
            sy = rtmp.tile([1, NT], F32, tag="rt")
            nc.scalar.activation(sy, rps, AF.Sqrt)
            t1 = rtmp.tile([1, NT], F32, tag="rt")
            nc.vector.reciprocal_approx_fast(t1, sy)
            t1h = rtmp.tile([1, NT], F16, tag="rth")
            nc.vector.tensor_copy(t1h, t1)
            st["ye8"][ti] = ye16
            st["t1h_%d" % ti] = t1h

        def y_pe(st, ti):
            # broadcast 1/(16||ye||) and reduce yn into ybar parts; emitted
            # after a dense matmul block so the PE never waits on the chain.
            ye16 = st["ye8"][ti]
            ivb = bcp.tile([128, NT], F32, tag="bc")
            nc.tensor.matmul(ivb, onesr16_sb, st["t1h_%d" % ti],
                             start=True, stop=True)
            # 16-bit SBUF copy so the yn ops run all-16-bit
            ivb16 = bc16p.tile([128, NT], F16, tag="bc16")
            nc.scalar.activation(ivb16, ivb, AF.Identity)
            for m in range(KD):
                tmp = tmpp.tile([128, NT], F16, tag="tmp")
                # fused: tmp = ye*invy, ybp[...] = sum_t tmp
                nc.vector.scalar_tensor_tensor(
                    tmp, ye16[:, m, :], 1.0, ivb16,
                    op0=ALU.mult, op1=ALU.mult,
                    accum_out=st["ybp"][:, m // 2, m % 2, ti:ti + 1])
            if ti == NTT - 1:
                b = st["b"]
                # ybf = sum_s yn = 1024*ybar; fp8 copy is exactly the 1024x
                # scale the q matmul wants
                ybf = bat.tile([128, KDP, 2, 1], F32, tag="ybf",
                               name=f"yf{b}")
                nc.vector.tensor_add(ybf, st["ybp"][:, :, :, 0:1],
                                     st["ybp"][:, :, :, 1:2])
                # padded [.., 2, 16] fp8 layout: dual-row ldweights needs the
                # k-pair step 16B-aligned
                ybar8 = bat.tile([128, KDP, 2, 16], F8, tag="ybar",
                                 name=f"yb{b}")
                nc.vector.tensor_copy(ybar8[:, :, :, 0:1], ybf)
                st["ybar8"] = ybar8
                if dbg and b == 0:
                    nc.sync.dma_start(dbg["yb"][:, :, :, :], ybar8)

        def pass2_q(st):
            # q = 16384*(xe.ybar) ; scores += (rw1/1024)*q*(1/(16||xe||))*16
            #   -> c2 = rw1/1024 folded into invx  (16*16*1024/16384 = 16/16=1)
            b = st["b"]
            mxp = rows.tile([1, NTT], F32, tag="mxp", name=f"mxp{b}")
            for ti in range(NTT):
                ns = slice(ti * NT, (ti + 1) * NT)
                qps = rpp.tile([1, NT], F32, tag="row")
                for kp in range(KDP):
                    nc.tensor.matmul(qps,
                                     st["ybar8"][:, kp, :, 0:1],
                                     st["xe8"][:, 2 * kp:2 * kp + 2, ns],
                                     start=(kp == 0), stop=(kp == KDP - 1),
                                     perf_mode=DR)
                s0 = rtmp.tile([1, NT], F32, tag="rt")
                nc.vector.tensor_mul(s0, qps, st["invx"][:, ns])
                nc.vector.tensor_add(st["scores"][:, ns], st["scores"][:, ns],
                                     s0)
                nc.vector.reduce_max(mxp[:, ti:ti + 1], st["scores"][:, ns],
                                     axis=AX.X)
            scores = st["scores"]
            mx = rows.tile([1, 1], F32, tag="mx", name=f"mx{b}")
            nc.vector.reduce_max(mx, mxp, axis=AX.X, negate=True)
            nc.scalar.activation(scores, scores, AF.Exp, bias=mx)
            ewh = rows.tile([1, T], F16, tag="ewh", name=f"ew{b}")
            nc.vector.tensor_copy(ewh, scores)
            st["ewh"] = ewh
            if dbg:
                nc.sync.dma_start(dbg["ew"][b:b + 1, :], ewh)
            # 1/sum lands at partition b of smcol; applied as a per-partition
            # ACT scale on the final [4, D] correction matmul
            sm = rows.tile([1, 1], F32, tag="sm", name=f"sm{b}")
            nc.vector.reduce_sum(sm, scores, axis=AX.X)
            smi = rows.tile([1, 1], F32, tag="smi", name=f"smi{b}")
            nc.vector.reciprocal(smi, sm)
            # DVE/ACT can't write unaligned partitions; DMA the scalar to
            # partition b of smcol
            nc.sync.dma_start(smcol[b:b + 1, :], smi)

        def pass2_w(st, tail=False):
            # V[:, :, b] = sum_t w[t] * h[:, t] on the DVE (hidden under the
            # next batch's matmuls); in the tail, alternate ACT accum_out and
            # DVE reduce, and interleave the per-k V writes so the final
            # correction matmuls stream behind the reduction pipeline.
            b = st["b"]
            oacc = bat.tile([128, KI, NTT], F32, tag="oacc", name=f"oa{b}")
            for ti in range(NTT):
                ns = slice(ti * NT, (ti + 1) * NT)
                wbc = bcp.tile([128, NT], F32, tag="bc")
                nc.tensor.matmul(wbc, onesr16_sb, st["ewh"][:, ns],
                                 start=True, stop=True)
                wbc16 = bc16p.tile([128, NT], F16, tag="bc16")
                nc.scalar.activation(wbc16, wbc, AF.Identity)
                for m in range(KI):
                    tmp = tmpp.tile([128, NT], F16, tag="tmp")
                    # fused: tmp = h*w, oacc[...] = sum_t tmp
                    nc.vector.scalar_tensor_tensor(
                        tmp, st["h"][:, m, ns], 1.0, wbc16,
                        op0=ALU.mult, op1=ALU.mult,
                        accum_out=oacc[:, m, ti:ti + 1])
                    if tail and ti == NTT - 1:
                        # per-k V write unblocks correction matmul k
                        nc.vector.tensor_add(v16[:, m, b:b + 1],
                                             oacc[:, m, 0:1],
                                             oacc[:, m, 1:2])
                        nc.vector.tensor_scalar_mul(v16l[:, m, b:b + 1],
                                                    v16[:, m, b:b + 1],
                                                    1.0 / 16.0)
            if not tail:
                nc.vector.tensor_add(v16[:, :, b:b + 1], oacc[:, :, 0:1],
                                     oacc[:, :, 1:2])
                nc.vector.tensor_scalar_mul(v16l[:, :, b:b + 1],
                                            v16[:, :, b:b + 1], 1.0 / 16.0)

        def final_correction():
            # out[b, :] = (V[:, b] @ (W2hi + W2lo/16)) / (64*sum_b) + b2
            #           = V[:, b] @ W2 / sum_b + b2
            HD = D // 2
            psc_t = [mmp.tile([128, NT], F32, tag="mm", name=f"pc{h}")
                     for h in range(2)]
            psc = [t[0:4, 0:HD] for t in psc_t]
            for k in range(KI):
                for h in range(2):
                    hs = slice(h * HD, (h + 1) * HD)
                    nc.tensor.matmul(psc[h], v16[:, k, :], w2h_sb[:, k, hs],
                                     start=(k == 0), stop=False)
                    nc.tensor.matmul(psc[h], v16l[:, k, :], w2l_sb[:, k, hs],
                                     start=False, stop=(k == KI - 1))
            sA = bat.tile([4, 1], F32, tag="sA", name="sA")
            nc.vector.tensor_scalar_mul(sA, smcol, 1.0 / SW)
            outf = bat.tile([4, D], F32, tag="outf", name="outf")
            for h in range(2):
                hs = slice(h * HD, (h + 1) * HD)
                nc.scalar.activation(outf[:, hs], psc[h], AF.Identity,
                                     scale=sA)
                nc.vector.tensor_add(outf[:, hs], outf[:, hs], b2r_sb[:, hs])
            nc.sync.dma_start(out[:, :], outf)
            if dbg:
                nc.sync.dma_start(dbg["v"][:, :, :], v16)
                nc.sync.dma_start(dbg["sm"][:, :], smcol)

        prev = None
        for b in range(bpc):
            st = alloc_batch(b)
            h8_0 = fc1_part(st, 0)
            if prev is not None:
                y_pe(prev, 1)            # chain computed during prev's pwy
            fc2_part(st, 0, h8_0)        # DR MMs cover prev's yn/ybar DVE
            if prev is not None:
                pass2_q(prev)            # q MMs + softmax chain on DVE/ACT
            une_part(st, 0)
            pwx_part(st, 0)
            if prev is not None:
                pass2_w(prev)            # weight-bcast + weighted-sum DVE
            pwy_part(st, 0)
            h8_1 = fc1_part(st, 1)
            if dbg and b == 0:
                nc.sync.dma_start(dbg["h"][:, :, :], st["h"])
            y_pe(st, 0)                  # chain from pwy(st,0) long ready
            fc2_part(st, 1, h8_1)
            if b < bpc - 1:
                une_part(st, 1)
                pwx_part(st, 1)
                if dbg and b == 0:
                    nc.sync.dma_start(dbg["xe8"][:, :, :], st["xe8"])
                pwy_part(st, 1)
            else:
                # last batch: start the invy chain as early as possible and
                # hide its own reduction tail behind its remaining matmuls
                pwy_part(st, 1)
                une_part(st, 1)
                y_pe(st, 1)
                pwx_part(st, 1)
            prev = st
        pass2_q(prev)
        pass2_w(prev, tail=True)
        final_correction()


_CACHE = {}


def _get_nc():
    if "nc" not in _CACHE:
        nc = build_nc(BPC)
        nc.finalize()
        _CACHE["nc"] = nc
    return _CACHE["nc"]


def _q8(a, scale):
    return (np.asarray(a, np.float32) * scale).astype(ml_dtypes.float8_e4m3)


def make_in_maps(x, fc1_w, fc1_b, fc2_w, fc2_b, un_emb_w, un_emb_b,
                 un_red_w, un_red_b, pw_x_w, pw_x_b, pw_y_w, pw_y_b, red_w):
    w2s = np.ascontiguousarray(fc2_w.T).astype(np.float32) * SW
    w2hi = w2s.astype(ml_dtypes.float8_e4m3)
    w2lo = ((w2s - w2hi.astype(np.float32)) * 16.0).astype(
        ml_dtypes.float8_e4m3)
    shared = {
        "w1t": np.ascontiguousarray(fc1_w.T).astype(np.float16),
        "w2hi": w2hi,
        "w2lo": w2lo,
        "wet": _q8(np.ascontiguousarray(un_emb_w.T), SW),
        "wxt": _q8(np.ascontiguousarray(pw_x_w.T), SW),
        "wyt": _q8(np.ascontiguousarray(pw_y_w.T), SW),
        "wrt": np.ascontiguousarray(un_red_w.T).astype(np.float16),
        "b1": np.asarray(fc1_b, np.float32),
        "b2s": np.asarray(fc2_b, np.float32) * SX,
        "be": np.asarray(un_emb_b, np.float32),
        "bxs": np.asarray(pw_x_b, np.float32) * SX,
        "bys": np.asarray(pw_y_b, np.float32) * SX,
        "consts": np.array([un_red_b[0] * red_w[0], red_w[0],
                            red_w[1] / 1024.0, 0.0], np.float32),
        "onesv": np.ones([128, 1], np.float16),
        "onesr16": np.ones([1, 128], np.float16),
        "b2rep": np.tile(np.asarray(fc2_b, np.float16)[None, :], (4, 1)),
    }
    in_maps = []
    for c in range(NCORES):
        xs = np.ascontiguousarray(
            x[c * BPC:(c + 1) * BPC].transpose(0, 2, 1)).astype(np.float16)
        in_maps.append({"xt": xs, **shared})
    return in_maps


def kernel(**inputs) -> np.ndarray:
    inputs = {k: np.asarray(v) for k, v in inputs.items()}
    nc = _get_nc()
    in_maps = make_in_maps(**inputs)
    res = run_bass_kernel_spmd(nc, in_maps, core_ids=list(range(NCORES)))
    return np.concatenate([res.results[c]["out"] for c in range(NCORES)], axis=0)
